# revision 1
# baseline (speedup 1.0000x reference)
"""GroupedEmbeddingBag kernel for 8 trn2 NeuronCores.

Table-parallel: core c handles table c (weights[c], values[c], offsets[c]).
Per core: 1600 indirect-DMA gathers (128 rows of 512B each) pull embedding
rows into SBUF in position order; TensorE matmuls with host-baked 0/1
selection matrices segment-sum them into PSUM "epoch" tiles (one epoch = 4
tiles = 512 positions, bag window W slots). Epoch results stream back to
DRAM; the host maps (epoch, slot) -> bag and concatenates tables.
"""

import sys

sys.path.insert(0, "/opt/trn_rl_repo")

import numpy as np

T, V, D, B = 8, 100000, 128, 4096
L = 204800
P = 128
NTILES = L // P            # 1600
EP_TILES = 4               # tiles per epoch
NEP = NTILES // EP_TILES   # 400 epochs
CHUNK_TILES = 16           # sel streaming chunk (4 epochs)
NCHUNK = NTILES // CHUNK_TILES
OUT_RING_EP = 4            # epochs per output DMA

_compiled = {}


def _patch_drain(tile_mod, mybir):
    from concourse.vector_clock import ScopedClock

    def _patched(self, tick_clock, wait_clock):
        # this walrus build allows only ONE sync-wait on the tail Drain:
        # spread the rest over preceding nops, one wait each.
        NNOPS = 64
        nops = [self.nc.sync.nop(nofuse=True, hint=f"dw_{i}") for i in range(NNOPS)]
        drain_inst = self.nc.sync.drain()
        wait_clock.add_sem_waits(
            drain_inst.ins, ScopedClock({None: tick_clock.global_clock})
        )
        dsi = drain_inst.ins.sync_info
        waits = list(dsi.on_wait) if dsi else []
        if len(waits) > 1:
            del dsi.on_wait[1:]
            rest = waits[1:]
            assert len(rest) <= NNOPS, f"too many drain waits: {len(waits)}"
            for nop, w in zip(nops, rest):
                nsi = nop.ins.sync_info
                if nsi is None:
                    nop.ins.sync_info = mybir.SyncInfo(on_wait=[w], on_update=[])
                else:
                    nsi.on_wait.append(w)
        self.nc.all_engine_barrier()
        popped = self.nc._tile_sem_poison_stack.pop()
        assert popped is self._sem_poison
        self.nc.clear_and_free_semaphores(list(self.sems.allocated().values()))
        self.nc.all_engine_barrier()

    tile_mod.TileContext._drain_and_barrier = _patched


def _split_waits(nc, mybir, maxw=1):
    # this walrus build rejects >1 sync-wait on an instruction: hoist extra
    # waits onto same-engine nops spliced in directly before it.
    cnt = 0
    for fn in nc.m.functions:
        for blk in fn.blocks:
            new_insts = []
            for inst in blk.instructions:
                si = inst.sync_info
                if si is not None and len(si.on_wait) > maxw:
                    extra = list(si.on_wait[maxw:])
                    del si.on_wait[maxw:]
                    for w in extra:
                        nop = mybir.InstNoOp(
                            name=f"waitnop-{cnt}", engine=inst.engine, ins=[], outs=[]
                        )
                        cnt += 1
                        nop.sync_info = mybir.SyncInfo(on_wait=[w], on_update=[])
                        new_insts.append(nop)
                new_insts.append(inst)
            blk.instructions[:] = new_insts
    return cnt


def _build(W):
    import concourse.bass as bass
    import concourse.mybir as mybir
    import concourse.tile as tile

    _patch_drain(tile, mybir)

    nc = bass.Bass()
    wt = nc.declare_dram_parameter("wt", [V, D], mybir.dt.float32, isOutput=False)
    vals = nc.declare_dram_parameter("vals", [P, NTILES], mybir.dt.int32, isOutput=False)
    sel = nc.declare_dram_parameter("sel", [P, NTILES * W], mybir.dt.float32, isOutput=False)
    oslots = nc.declare_dram_parameter("oslots", [W, NEP * D], mybir.dt.float32, isOutput=True)

    with tile.TileContext(nc) as tc:
        with (
            tc.tile_pool(name="valsp", bufs=1) as valsp,
            tc.tile_pool(name="selp", bufs=3) as selp,
            tc.tile_pool(name="ep", bufs=8) as ep,
            tc.tile_pool(name="outp", bufs=2) as outp,
            tc.tile_pool(name="psum", bufs=8, space="PSUM") as psump,
        ):
            vals_sb = valsp.tile([P, NTILES], mybir.dt.int32)
            nc.sync.dma_start(out=vals_sb[:], in_=vals[:])
            out_ring = None
            psum_t = None
            for c in range(NCHUNK):
                sel_sb = selp.tile([P, CHUNK_TILES * W], mybir.dt.float32, tag="sel")
                nc.sync.dma_start(
                    out=sel_sb[:], in_=sel[:, c * CHUNK_TILES * W:(c + 1) * CHUNK_TILES * W]
                )
                for tl in range(CHUNK_TILES):
                    t = c * CHUNK_TILES + tl
                    e = t // EP_TILES
                    ph = t % EP_TILES
                    et = ep.tile([P, D], mybir.dt.float32, tag="e")
                    nc.gpsimd.indirect_dma_start(
                        out=et[:],
                        out_offset=None,
                        in_=wt[:],
                        in_offset=bass.IndirectOffsetOnAxis(
                            ap=vals_sb[:, t:t + 1], axis=0
                        ),
                    )
                    if ph == 0:
                        psum_t = psump.tile([W, D], mybir.dt.float32, tag="ps")
                    nc.tensor.matmul(
                        out=psum_t[:],
                        lhsT=sel_sb[:, tl * W:(tl + 1) * W],
                        rhs=et[:],
                        start=(ph == 0),
                        stop=(ph == EP_TILES - 1),
                    )
                    if ph == EP_TILES - 1:
                        er = e % OUT_RING_EP
                        if er == 0:
                            out_ring = outp.tile([W, OUT_RING_EP * D], mybir.dt.float32, tag="or")
                        nc.vector.tensor_copy(
                            out=out_ring[:, er * D:(er + 1) * D], in_=psum_t[:]
                        )
                        if er == OUT_RING_EP - 1:
                            e0 = e - (OUT_RING_EP - 1)
                            nc.sync.dma_start(
                                out=oslots[:, e0 * D:(e0 + OUT_RING_EP) * D],
                                in_=out_ring[:],
                            )
    _split_waits(nc, mybir)
    return nc


def kernel(values, offsets, weights):
    from concourse.bass_utils import run_bass_kernel_spmd

    values = np.asarray(values)
    offsets = np.asarray(offsets)
    weights = np.ascontiguousarray(np.asarray(weights, dtype=np.float32))

    pos = np.arange(L)
    seg = np.empty((T, L), dtype=np.int64)
    for c in range(T):
        seg[c] = np.searchsorted(offsets[c, 1:], pos, side="right")

    # epoch windows: epoch e covers positions [512e, 512e+512)
    segr = seg.reshape(T, NEP, EP_TILES * P)
    b_lo = segr[:, :, 0]                      # [T, NEP]
    b_hi = segr[:, :, -1]
    S = (b_hi - b_lo + 1).astype(np.int64)    # slots used per epoch
    W = int(S.max())
    W = max(4, (W + 3) // 4 * 4)
    assert W <= 128, f"epoch bag-window {W} exceeds PSUM partition limit"

    in_maps = []
    for c in range(T):
        vals_t = np.ascontiguousarray(
            values[c].reshape(NTILES, P).T.astype(np.int32)
        )  # [P, NTILES]
        # sel[j, t, s] = 1 if seg[128t + j] == b_lo[e(t)] + s
        seg_l = seg[c].reshape(NTILES, P).T.astype(np.int32)  # [P, NTILES]
        base = np.repeat(b_lo[c], EP_TILES).astype(np.int32)  # [NTILES]
        loc = seg_l - base[None, :]                           # [P, NTILES]
        selm = (loc[:, :, None] == np.arange(W, dtype=np.int32)[None, None, :])
        sel = np.ascontiguousarray(
            selm.reshape(P, NTILES * W).astype(np.float32)
        )
        in_maps.append({"wt": weights[c], "vals": vals_t, "sel": sel})

    key = W
    if key not in _compiled:
        _compiled.clear()
        _compiled[key] = _build(W)
    nc = _compiled[key]

    global _last_inmaps
    _last_inmaps = in_maps
    res = run_bass_kernel_spmd(nc, in_maps, core_ids=list(range(T)))

    out = np.zeros((B, T * D), dtype=np.float32)
    for c in range(T):
        osl = res.results[c]["oslots"].reshape(W, NEP, D)
        pooled = np.zeros((B, D), dtype=np.float32)
        for e in range(NEP):
            lo = int(b_lo[c, e])
            n = int(S[c, e])
            pooled[lo:lo + n] += osl[:n, e, :]
        out[:, c * D:(c + 1) * D] = pooled
    return out


if __name__ == "__main__":
    rng = np.random.default_rng(0)
    values = rng.integers(0, V, size=(T, L)).astype(np.int64)
    inner = np.sort(rng.integers(0, L, size=(T, B - 1)), axis=1)
    offsets = np.concatenate(
        [np.zeros((T, 1), np.int64), inner, np.full((T, 1), L, np.int64)], axis=1
    )
    weights = (rng.standard_normal((T, V, D)) * 0.01).astype(np.float32)
    out = kernel(values, offsets, weights)
    # numpy reference
    exp = np.zeros((B, T * D), dtype=np.float32)
    for c in range(T):
        pooled = np.zeros((B, D), np.float32)
        np.add.at(pooled, np.searchsorted(offsets[c, 1:], np.arange(L), side="right"), weights[c][values[c]])
        exp[:, c * D:(c + 1) * D] = pooled
    err = np.linalg.norm(out - exp) / np.linalg.norm(exp)
    print("self-check rel err:", err)



# revision 10
# speedup vs baseline: 1.8797x; 1.8797x over previous
"""GroupedEmbeddingBag kernel for 8 trn2 NeuronCores.

Table-parallel: core c handles table c (weights[c], values[c], offsets[c]).

Per core the id stream is split into 16 position-chunks; within a chunk ids
are bucket-sorted into 4 contiguous table-row ranges of 25000 rows so that
dma_gather (InstDMAGatherAnt, int16 relative indices) can pull thousands of
rows per SWDGE instruction — the per-instruction descriptor-generation
overhead that dominated an indirect_dma_start-per-tile design is amortized
away. Gathered rows land in SBUF in (chunk, bucket, position) slot order;
the host knows the exact slot layout, so it ships a per-slot bag-id byte
(epoch-relative). The DVE expands bag-ids into one-hot bf16 selection
matrices on-chip (is_equal vs an iota constant), TensorE matmuls
segment-sum 4-tile epochs into PSUM bag windows, the Scalar engine copies
finished windows to an SBUF ring, and windows stream back to DRAM as bf16.
The host overlap-adds the epoch windows into the final pooled output.

Weights are bf16 (PSUM accumulation fp32): rel-err vs the fp32 reference
~2e-3, far inside the 2e-2 gate.
"""

import os
import sys

sys.path.insert(0, "/opt/trn_rl_repo")

import numpy as np
import ml_dtypes

T, V, D, B = 8, 100000, 128, 4096
L = 204800
P = 128
NB = 4                      # index-range buckets (int16 addressing limit)
BROWS = V // NB             # 25000 rows per bucket
NCH = 16                    # position chunks
CHP = L // NCH              # 12800 positions per chunk
EP_T = 4                    # slot-tiles per PSUM epoch
OUT_RING_EP = 8             # epochs per output DMA

_compiled = {}


def _patch_drain(tile_mod, mybir):
    from concourse.vector_clock import ScopedClock

    def _patched(self, tick_clock, wait_clock):
        # this walrus build allows only ONE sync-wait on the tail Drain:
        # spread the rest over preceding nops, one wait each.
        NNOPS = 64
        nops = [self.nc.sync.nop(nofuse=True, hint=f"dw_{i}") for i in range(NNOPS)]
        drain_inst = self.nc.sync.drain()
        wait_clock.add_sem_waits(
            drain_inst.ins, ScopedClock({None: tick_clock.global_clock})
        )
        dsi = drain_inst.ins.sync_info
        waits = list(dsi.on_wait) if dsi else []
        if len(waits) > 1:
            del dsi.on_wait[1:]
            rest = waits[1:]
            assert len(rest) <= NNOPS, f"too many drain waits: {len(waits)}"
            for nop, w in zip(nops, rest):
                nsi = nop.ins.sync_info
                if nsi is None:
                    nop.ins.sync_info = mybir.SyncInfo(on_wait=[w], on_update=[])
                else:
                    nsi.on_wait.append(w)
        self.nc.all_engine_barrier()
        popped = self.nc._tile_sem_poison_stack.pop()
        assert popped is self._sem_poison
        self.nc.clear_and_free_semaphores(list(self.sems.allocated().values()))
        self.nc.all_engine_barrier()

    tile_mod.TileContext._drain_and_barrier = _patched


def _split_waits(nc, mybir, maxw=1):
    # this walrus build rejects >1 sync-wait on an instruction: hoist extra
    # waits onto same-engine nops spliced in directly before it.
    cnt = 0
    for fn in nc.m.functions:
        for blk in fn.blocks:
            new_insts = []
            for inst in blk.instructions:
                si = inst.sync_info
                if si is not None and len(si.on_wait) > maxw:
                    extra = list(si.on_wait[maxw:])
                    del si.on_wait[maxw:]
                    for w in extra:
                        nop = mybir.InstNoOp(
                            name=f"waitnop-{cnt}", engine=inst.engine, ins=[], outs=[]
                        )
                        cnt += 1
                        nop.sync_info = mybir.SyncInfo(on_wait=[w], on_update=[])
                        new_insts.append(nop)
                new_insts.append(inst)
            blk.instructions[:] = new_insts
    return cnt


def _build(W, S):
    """W: epoch bag-window (PSUM partitions). S: padded slots per
    (chunk, bucket), multiple of 512."""
    import concourse.bass as bass
    import concourse.mybir as mybir
    import concourse.tile as tile
    from concourse import library_config, library_overlay

    _patch_drain(tile, mybir)

    ST_B = S // P               # slot-tiles per bucket segment
    NT_CH = NB * ST_B           # slot-tiles per chunk
    NTT = NCH * NT_CH           # total slot-tiles
    NEPO = NTT // EP_T          # total epochs
    SC = S // 16                # idx columns per (chunk, bucket)

    bf16 = mybir.dt.bfloat16

    nc = bass.Bass()
    wt = nc.declare_dram_parameter("wt", [V, D], bf16, isOutput=False)
    idxs = nc.declare_dram_parameter("idxs", [P, NCH * NB * SC], mybir.dt.int16, isOutput=False)
    cnts = nc.declare_dram_parameter("cnts", [1, NCH * NB], mybir.dt.int32, isOutput=False)
    bagid = nc.declare_dram_parameter("bagid", [P, NTT], mybir.dt.int8, isOutput=False)
    iota = nc.declare_dram_parameter("iota", [P, W], mybir.dt.int8, isOutput=False)
    oslots = nc.declare_dram_parameter("oslots", [W, NEPO * D], bf16, isOutput=True)

    with tile.TileContext(nc) as tc:
        nc.gpsimd.load_library(library_config.mlp)
        with (
            tc.tile_pool(name="inp", bufs=1) as inp,
            tc.tile_pool(name="selp", bufs=2) as selp,
            tc.tile_pool(name="ep", bufs=2) as ep,
            tc.tile_pool(name="outp", bufs=2) as outp,
            tc.tile_pool(name="psum", bufs=8, space="PSUM") as psump,
        ):
            idxs_sb = inp.tile([P, NCH * NB * SC], mybir.dt.int16)
            nc.sync.dma_start(out=idxs_sb[:], in_=idxs[:])
            cnts_sb = inp.tile([1, NCH * NB], mybir.dt.int32)
            nc.sync.dma_start(out=cnts_sb[:], in_=cnts[:])
            bagid_sb = inp.tile([P, NTT], mybir.dt.int8)
            nc.sync.dma_start(out=bagid_sb[:], in_=bagid[:])
            iota_sb = inp.tile([P, W], mybir.dt.int8)
            nc.sync.dma_start(out=iota_sb[:], in_=iota[:])

            cregs = [nc.gpsimd.alloc_register(name=f"creg{b}") for b in range(NB)]

            out_ring = None
            psum_t = None
            for k in range(NCH):
                et = ep.tile([P, NT_CH * D], bf16, tag="e")
                if k < 2:
                    # ring buffers start with arbitrary SBUF bits; pad slots
                    # (idx -1 -> no DMA write) must stay finite since 0*NaN
                    # poisons PSUM. After the first two chunks the buffers
                    # only ever hold stale gathered rows (finite).
                    nc.vector.memset(et[:], 0)
                for b in range(NB):
                    i = k * NB + b
                    nc.gpsimd.load(cregs[b], cnts_sb[:1, i:i + 1])
                    nc.gpsimd.dma_gather(
                        et[:, b * ST_B * D:(b + 1) * ST_B * D].rearrange(
                            "p (s d) -> p s d", d=D
                        ),
                        wt[b * BROWS:(b + 1) * BROWS, :],
                        idxs_sb[:, i * SC:(i + 1) * SC],
                        S,
                        cregs[b],
                        D,
                        single_packet=False,
                    )
                sel_sb = selp.tile([P, NT_CH * W], bf16, tag="sel")
                nc.vector.tensor_tensor(
                    out=sel_sb[:].rearrange("p (t w) -> p t w", w=W),
                    in0=bagid_sb[:, k * NT_CH:(k + 1) * NT_CH]
                    .rearrange("p t -> p t ()")
                    .to_broadcast([P, NT_CH, W]),
                    in1=iota_sb[:]
                    .rearrange("p w -> p () w")
                    .to_broadcast([P, NT_CH, W]),
                    op=mybir.AluOpType.is_equal,
                )
                for tl in range(NT_CH):
                    st = k * NT_CH + tl
                    e = st // EP_T
                    ph = st % EP_T
                    if ph == 0:
                        psum_t = psump.tile([W, D], mybir.dt.float32, tag="ps")
                    nc.tensor.matmul(
                        out=psum_t[:],
                        lhsT=sel_sb[:, tl * W:(tl + 1) * W],
                        rhs=et[:, tl * D:(tl + 1) * D],
                        start=(ph == 0),
                        stop=(ph == EP_T - 1),
                    )
                    if ph == EP_T - 1:
                        er = e % OUT_RING_EP
                        if er == 0:
                            out_ring = outp.tile([W, OUT_RING_EP * D], bf16, tag="or")
                        nc.scalar.copy(
                            out=out_ring[:, er * D:(er + 1) * D], in_=psum_t[:]
                        )
                        if er == OUT_RING_EP - 1:
                            e0 = e - (OUT_RING_EP - 1)
                            nc.sync.dma_start(
                                out=oslots[:, e0 * D:(e0 + OUT_RING_EP) * D],
                                in_=out_ring[:],
                            )
    _split_waits(nc, mybir)
    library_overlay.lower_extended_insts(nc)
    return nc


def _prep_core(values_c, seg_c, S):
    """Slot layout for one table. Returns idxs [P, NCH*NB*S/16] int16,
    cnts [NCH*NB] int32, slot_seg [NCH*NB*S] int64 (-1 for pad slots)."""
    SC = S // 16
    idxs = np.zeros((16, NCH * NB * SC), np.int16)
    cnts = np.zeros(NCH * NB, np.int32)
    slot_seg = np.full(NCH * NB * S, -1, np.int64)
    bucket = values_c // BROWS
    for k in range(NCH):
        vpos = np.arange(k * CHP, (k + 1) * CHP)
        vb = bucket[vpos]
        for b in range(NB):
            i = k * NB + b
            pos_b = vpos[vb == b]           # position order preserved
            n = len(pos_b)
            assert n <= S, f"bucket overflow {n} > {S}"
            rel = np.full(S, -1, np.int16)
            rel[:n] = (values_c[pos_b] - b * BROWS).astype(np.int16)
            if n == 0:
                rel[0] = 0
                n = 1
            cnts[i] = n
            idxs[:, i * SC:(i + 1) * SC] = rel.reshape(SC, 16).T
            slot_seg[i * S:i * S + len(pos_b)] = seg_c[pos_b]
    idxs_full = np.tile(idxs, (8, 1))       # replicate across Q7 core groups
    return np.ascontiguousarray(idxs_full), cnts, slot_seg


def kernel(values, offsets, weights):
    from concourse.bass_utils import run_bass_kernel_spmd

    values = np.asarray(values)
    offsets = np.asarray(offsets)
    weights = np.ascontiguousarray(np.asarray(weights, dtype=np.float32))
    wts = weights.astype(ml_dtypes.bfloat16)

    pos = np.arange(L)
    seg = np.empty((T, L), dtype=np.int64)
    for c in range(T):
        seg[c] = np.searchsorted(offsets[c, 1:], pos, side="right")

    # padded slots per (chunk, bucket): multiple of 512 covering max count
    bucket = values // BROWS
    maxc = 0
    for c in range(T):
        for k in range(NCH):
            bc = np.bincount(bucket[c, k * CHP:(k + 1) * CHP], minlength=NB)
            maxc = max(maxc, int(bc.max()))
    S = ((maxc + 511) // 512) * 512

    ST_B = S // P
    NT_CH = NB * ST_B
    NTT = NCH * NT_CH
    NEPO = NTT // EP_T

    prep = [_prep_core(values[c], seg[c], S) for c in range(T)]

    # epoch windows over slot order; W = max bag span
    slot_seg = np.stack([p[2] for p in prep])          # [T, NTT*P]
    ss = slot_seg.reshape(T, NEPO, EP_T * P)
    ssm = np.ma.masked_equal(ss, -1)
    lo = ssm.min(axis=2).filled(0).astype(np.int64)    # [T, NEPO]
    hi = ssm.max(axis=2).filled(-1).astype(np.int64)
    span = np.maximum(hi - lo + 1, 0)
    W = int(span.max())
    W = max(4, (W + 3) // 4 * 4)
    assert W <= 128, f"epoch bag-window {W} exceeds PSUM partition limit"

    iota_np = np.tile(np.arange(W, dtype=np.int8), (P, 1))

    in_maps = []
    for c in range(T):
        idxs_c, cnts_c, sseg_c = prep[c]
        # bag-id byte per slot: seg - lo(epoch), pad slots -> -1
        ep_lo = np.repeat(lo[c], EP_T * P)
        bid = np.where(sseg_c >= 0, sseg_c - ep_lo, -1).astype(np.int8)
        bid_tile = np.ascontiguousarray(bid.reshape(NTT, P).T)  # [P, NTT]
        in_maps.append({
            "wt": wts[c],
            "idxs": idxs_c,
            "cnts": cnts_c.reshape(1, -1),
            "bagid": bid_tile,
            "iota": iota_np,
        })

    key = (W, S)
    if key not in _compiled:
        _compiled.clear()
        _compiled[key] = _build(W, S)
    nc = _compiled[key]

    global _last_inmaps
    _last_inmaps = in_maps
    res = run_bass_kernel_spmd(nc, in_maps, core_ids=list(range(T)))

    out = np.zeros((B, T * D), dtype=np.float32)
    for c in range(T):
        osl = np.asarray(res.results[c]["oslots"], dtype=np.float32).reshape(W, NEPO, D)
        pooled = np.zeros((B, D), dtype=np.float32)
        for e in range(NEPO):
            n = int(span[c, e])
            if n == 0:
                continue
            lo_e = int(lo[c, e])
            pooled[lo_e:lo_e + n] += osl[:n, e, :]
        out[:, c * D:(c + 1) * D] = pooled
    return out


if __name__ == "__main__":
    rng = np.random.default_rng(0)
    values = rng.integers(0, V, size=(T, L)).astype(np.int64)
    inner = np.sort(rng.integers(0, L, size=(T, B - 1)), axis=1)
    offsets = np.concatenate(
        [np.zeros((T, 1), np.int64), inner, np.full((T, 1), L, np.int64)], axis=1
    )
    weights = (rng.standard_normal((T, V, D)) * 0.01).astype(np.float32)
    out = kernel(values, offsets, weights)
    exp = np.zeros((B, T * D), dtype=np.float32)
    for c in range(T):
        pooled = np.zeros((B, D), np.float32)
        np.add.at(pooled, np.searchsorted(offsets[c, 1:], np.arange(L), side="right"), weights[c][values[c]])
        exp[:, c * D:(c + 1) * D] = pooled
    err = np.linalg.norm(out - exp) / np.linalg.norm(exp)
    print("self-check rel err:", err)
